# revision 27
# baseline (speedup 1.0000x reference)
"""GAT graph classifier (3 GATConv layers + global mean pool + linear) on 8
Trainium2 NeuronCores.

Sharding: nodes (and their incident edges, by destination) are partitioned
across the 8 cores; each core gathers source features from a replicated
(all-gathered) node-feature table, does the segment-softmax + weighted
aggregation for its destination nodes via one-hot matmuls accumulated in PSUM,
then computes the next layer's projection for its own nodes; the small weight
matrices are replicated.

v2 structure (vs v1): the per-edge a_d gather is gone — a_d for own nodes
stays resident in SBUF and is broadcast to edge slots with a one-hot matmul
(lhsT = host-streamed S0^T tiles). The scatter one-hot S0 is built on-chip
from a resident dst-slot table via is_equal. Feature gathers round-robin
across SWDGE queues 1-3, which generate descriptors concurrently (~2.2x the
single-queue rate). Internal feature layout for layers 1-2 is "R" (c-major):
column c*8+h holds head h channel c, making the per-edge softmax-weight
broadcast multiply a regular strided access pattern.
"""
import sys
sys.path.insert(0, '/opt/trn_rl_repo')
import math
import numpy as np
from contextlib import ExitStack

import concourse.bass as bass
import concourse.mybir as mybir
import concourse.tile as tile
from concourse import bacc
from concourse.masks import make_identity

P = 128
F16 = mybir.dt.float16
F32 = mybir.dt.float32
I16 = mybir.dt.int16
NEG_SLOPE = 0.2
EPS = 1e-12
NQ = 3  # SWDGE queues 1..3 for gathers
NUM_G = 64


class Cfg:
    def __init__(self, N=50000, E=800000, F=128, HID=16, H=8, OUT=10, G=64,
                 CORES=8, SPLIT=32768, CB=2):
        self.N, self.E, self.F, self.HID, self.H, self.OUT, self.G = N, E, F, HID, H, OUT, G
        self.CORES, self.SPLIT, self.CB = CORES, SPLIT, CB
        self.D = HID * H                      # 128
        self.NSH = N // CORES                 # nodes per core
        self.NBLK = math.ceil(self.NSH / P)   # dst blocks per core
        assert N % CORES == 0


def _wrap_idx(flat):
    """Pack a flat int index list (len = multiple of 128) into the
    [128, len/16] int16 layout dma_gather expects (16-partition wrap,
    replicated to 8 groups of 16 partitions)."""
    n = len(flat)
    assert n % 128 == 0
    a = np.asarray(flat, np.int16).reshape(n // 16, 16).T   # [16, n/16]
    return np.tile(a, (8, 1))                               # [128, n/16]


def preprocess(edge_index, batch, cfg):
    """Host-side index-only preprocessing: shard edges by destination core,
    group by (dst block, src half), pad each group to 128-edge tiles with a
    tile count that is uniform across cores (SPMD: one program for all)."""
    c = cfg
    src = np.concatenate([np.asarray(edge_index[0]), np.arange(c.N)]).astype(np.int64)
    dst = np.concatenate([np.asarray(edge_index[1]), np.arange(c.N)]).astype(np.int64)

    # src half-table remap: node u -> (half, c*HSH + r') where r' = r % HSH
    # (half A = first HSH rows of each core's shard, B = rest)
    HSH = c.NSH // 2
    s_core, s_r = src // c.NSH, src % c.NSH
    s_half = (s_r >= HSH).astype(np.int64)
    s_idx = s_core * HSH + (s_r % HSH)

    per = [[[None, None] for _ in range(c.NBLK)] for _ in range(c.CORES)]
    core_of = dst // c.NSH
    for ci in range(c.CORES):
        m = core_of == ci
        es, ed, eh = s_idx[m], dst[m] - ci * c.NSH, s_half[m]
        blk = ed // P
        for b in range(c.NBLK):
            bm = blk == b
            for h in (0, 1):
                hm = bm & (eh == h)
                per[ci][b][h] = (es[hm].astype(np.int64), (ed[hm] % P).astype(np.int64))

    # uniform tile counts: max over cores per (block, half)
    T = np.zeros((c.NBLK, 2), np.int64)
    for b in range(c.NBLK):
        for h in range(2):
            mx = max(len(per[ci][b][h][0]) for ci in range(c.CORES))
            T[b, h] = max(1, math.ceil(mx / P))

    # chunks of CB blocks; per chunk tile order: all lo tiles (blocks in
    # order) then all hi tiles
    chunks = []
    t_all = 0
    col_lo = col_hi = 0
    for k0 in range(0, c.NBLK, c.CB):
        bs = list(range(k0, min(k0 + c.CB, c.NBLK)))
        Tlo = int(sum(T[b, 0] for b in bs))
        Thi = int(sum(T[b, 1] for b in bs))
        binfo = []
        tl = 0
        for b in bs:
            binfo.append({'b': b, 'lo': (tl, int(T[b, 0]))})
            tl += T[b, 0]
        th = Tlo
        for i, b in enumerate(bs):
            binfo[i]['hi'] = (int(th), int(T[b, 1]))
            th += T[b, 1]
        chunks.append({'blocks': binfo, 'Tlo': Tlo, 'Thi': Thi, 'T': Tlo + Thi,
                       'col_lo': col_lo, 'col_hi': col_hi, 'col_dst': t_all})
        col_lo += 8 * Tlo
        col_hi += 8 * Thi
        t_all += Tlo + Thi
    meta = {'chunks': chunks, 'T_ALL': t_all, 'COLS_LO': col_lo, 'COLS_HI': col_hi}

    # per-core arrays
    arrays = []
    for ci in range(c.CORES):
        ilo, ihi, dl = [], [], []
        for ch in chunks:
            for bi in ch['blocks']:
                b = bi['b']
                for h, acc_idx in ((0, ilo), (1, ihi)):
                    es, edl = per[ci][b][h]
                    n_pad = int(T[b, h]) * P - len(es)
                    acc_idx.extend(es.tolist())
                    acc_idx.extend([0] * n_pad)
            # dstloc in tile order (lo tiles then hi tiles), -1 pad
            for bi in ch['blocks']:
                b = bi['b']
                edl = per[ci][b][0][1]
                n_pad = int(T[b, 0]) * P - len(edl)
                dl.extend(edl.tolist())
                dl.extend([-1] * n_pad)
            for bi in ch['blocks']:
                b = bi['b']
                edl = per[ci][b][1][1]
                n_pad = int(T[b, 1]) * P - len(edl)
                dl.extend(edl.tolist())
                dl.extend([-1] * n_pad)
        dla = np.asarray(dl, np.int64).reshape(meta['T_ALL'], P)  # [T, p]
        # S0^T tiles, [128, T*128]: s0T[m, t*128+p] = (dla[t,p] == m);
        # S0 tiles, [128, T*128]: s0[p, t*128+m] = (dla[t,p] == m)
        s0t = (dla[None, :, :] == np.arange(P)[:, None, None]).astype(np.float16)
        arr = {
            'idx_lo': _wrap_idx(ilo) if ilo else np.zeros((P, 1), np.int16),
            'idx_hi': _wrap_idx(ihi) if ihi else np.zeros((P, 1), np.int16),
            's0t': s0t.reshape(P, meta['T_ALL'] * P).copy(),
            's0': s0t.transpose(2, 1, 0).reshape(P, meta['T_ALL'] * P).copy(),
        }
        # gsel [128, NBLK*G]: one-hot graph selector per block (pad rows zero)
        bl = np.full((P, c.NBLK), -1, np.int64)
        bb = np.asarray(batch)[ci * c.NSH:(ci + 1) * c.NSH]
        for b in range(c.NBLK):
            nrows = min(P, c.NSH - b * P)
            bl[:nrows, b] = bb[b * P:b * P + nrows]
        arr['gsel'] = (bl[:, :, None] == np.arange(NUM_G)[None, None, :]
                       ).astype(np.float16).reshape(P, c.NBLK * NUM_G)
        arrays.append(arr)
    return meta, arrays


def _sigma_perm(c):
    """R-permutation: h-major index f=h*16+c -> R index c*8+h."""
    f = np.arange(c.D)
    return (f % c.HID) * c.H + f // c.HID


def build_program(nc, cfg, meta, nonzero_bias):
    c = cfg
    ch_list = meta['chunks']
    ROW12 = 256               # fp16 cols per table row, layers 1-2 (512B)
    ROW3 = 128                # fp16 cols per table row, layer 3 (256B)

    # ---------------- kernel I/O ----------------
    x_shard = nc.declare_dram_parameter("x_shard", [c.NSH, c.F], F32, isOutput=False)
    W1 = nc.declare_dram_parameter("W1", [c.F, c.D], F32, isOutput=False)
    W2 = nc.declare_dram_parameter("W2", [c.D, c.D], F32, isOutput=False)
    W3 = nc.declare_dram_parameter("W3", [c.D, c.HID], F32, isOutput=False)
    atts = {}
    for l, hh in ((1, c.H), (2, c.H), (3, 1)):
        atts[l] = (
            nc.declare_dram_parameter(f"att_src{l}", [hh, c.HID], F32, isOutput=False),
            nc.declare_dram_parameter(f"att_dst{l}", [hh, c.HID], F32, isOutput=False),
        )
    b1 = nc.declare_dram_parameter("b1", [1, c.D], F32, isOutput=False)
    b2 = nc.declare_dram_parameter("b2", [1, c.D], F32, isOutput=False)
    b3 = nc.declare_dram_parameter("b3", [1, c.HID], F32, isOutput=False)
    W_lin = nc.declare_dram_parameter("W_lin", [c.HID, c.OUT], F32, isOutput=False)
    b_lin = nc.declare_dram_parameter("b_lin", [1, c.OUT], F32, isOutput=False)
    idx_lo = nc.declare_dram_parameter("idx_lo", [P, max(1, meta['COLS_LO'])], I16, isOutput=False)
    idx_hi = nc.declare_dram_parameter("idx_hi", [P, max(1, meta['COLS_HI'])], I16, isOutput=False)
    s0t_d = nc.declare_dram_parameter("s0t", [P, meta['T_ALL'] * P], F16, isOutput=False)
    s0_d = nc.declare_dram_parameter("s0", [P, meta['T_ALL'] * P], F16, isOutput=False)
    gsel_d = nc.declare_dram_parameter("gsel", [P, c.NBLK * c.G], F16, isOutput=False)
    iota128 = nc.declare_dram_parameter("iota128", [1, P], F16, isOutput=False)
    iota64 = nc.declare_dram_parameter("iota64", [1, c.G], F16, isOutput=False)
    sigma = nc.declare_dram_parameter("sigma", [P, 1], F32, isOutput=False)
    out = nc.declare_dram_parameter("out", [c.G, c.OUT], F32, isOutput=True)

    # ---------------- internal DRAM ----------------
    agin = {l: nc.dram_tensor(f"agin{l}", [c.NSH, ROW12 if l < 3 else ROW3], F16)
            for l in (1, 2, 3)}
    HSH = c.NSH // 2
    HN = c.N // 2
    xltab = {(l, h): nc.dram_tensor(f"xl{l}{'ab'[h]}",
                                    [HN, ROW12 if l < 3 else ROW3], F16,
                                    addr_space="Shared")
             for l in (1, 2, 3) for h in (0, 1)}
    arin = nc.dram_tensor("arin", [c.G, c.HID + 1], F32)
    arout = nc.dram_tensor("arout", [c.G, c.HID + 1], F32, addr_space="Shared")

    rg = [list(range(c.CORES))]

    with ExitStack() as ctx:
        tc = ctx.enter_context(tile.TileContext(nc))
        cpool = ctx.enter_context(tc.tile_pool(name="consts", bufs=1))
        wpool = ctx.enter_context(tc.tile_pool(name="wbuild", bufs=2))
        gpool = ctx.enter_context(tc.tile_pool(name="gath", bufs=4))
        ipool = ctx.enter_context(tc.tile_pool(name="idx", bufs=4))
        epool = ctx.enter_context(tc.tile_pool(name="estage", bufs=4))
        spool = ctx.enter_context(tc.tile_pool(name="s0", bufs=3))
        rpool = ctx.enter_context(tc.tile_pool(name="rhs", bufs=3))
        opool = ctx.enter_context(tc.tile_pool(name="post", bufs=3))
        # PSUM budget (8 banks): accs x2, ad x2, p1xl x2, misc x1, pooled x1
        ppA = ctx.enter_context(tc.tile_pool(name="psA", bufs=2, space="PSUM"))
        ppAd = ctx.enter_context(tc.tile_pool(name="psAd", bufs=3, space="PSUM"))
        ppB = ctx.enter_context(tc.tile_pool(name="psB", bufs=2, space="PSUM"))
        ppC = ctx.enter_context(tc.tile_pool(name="psC", bufs=1, space="PSUM"))

        # ---------------- constants ----------------
        IOTA = cpool.tile([P, P], F16)
        nc.sync.dma_start(out=IOTA[:], in_=iota128[0:1, :].to_broadcast([P, P]))
        ident32 = cpool.tile([c.G, c.G], F32)
        make_identity(nc, ident32[:])
        ones16 = cpool.tile([P, 1], F16)
        nc.gpsimd.memset(ones16[:], 1.0)
        sig_t = cpool.tile([P, 1], F32)
        nc.sync.dma_start(out=sig_t[:], in_=sigma[:, :])
        # row-permutation one-hot (lhsT): PsigT[k, r] = (r == sigma[k])
        PsigT = cpool.tile([P, P], F16)
        nc.vector.tensor_scalar(out=PsigT[:], in0=IOTA[:], scalar1=sig_t[:, 0:1],
                                scalar2=None, op0=mybir.AluOpType.is_equal)
        # a_d tables for own nodes, one per layer (written by phase 1)
        adS = {l: cpool.tile([P, c.NBLK * 8], F16, tag=f"adS{l}",
                             name=f"adS{l}") for l in (1, 2, 3)}

        # ---------------- W_aug builders ----------------
        def build_waug(l, Wp, Fin, Dcol, hh):
            """Returns fp16 [Fin, Dcol + 2*hh] tile: [W cols (R for l<3) |
            w_as | w_ad], rows R-permuted for l >= 2."""
            Wt = wpool.tile([Fin, Dcol], F32, tag="wld")
            nc.sync.dma_start(out=Wt[:], in_=Wp[:, :])
            aug = wpool.tile([Fin, Dcol + 2 * hh], F16, tag="waug")
            if l < 3:
                nc.vector.tensor_copy(
                    out=aug[:, 0:Dcol].rearrange("p (ch h) -> p ch h", ch=c.HID),
                    in_=Wt[:].rearrange("p (h ch) -> p h ch", h=c.H).transpose([0, 2, 1]),
                )
            else:
                nc.vector.tensor_copy(out=aug[:, 0:Dcol], in_=Wt[:])
            for j, att in enumerate(atts[l]):
                ab = wpool.tile([Fin, Dcol], F32, tag="attb")
                nc.sync.dma_start(
                    out=ab[:],
                    in_=att.ap().flatten().unsqueeze(0).to_broadcast([Fin, Dcol]))
                tmp = wpool.tile([Fin, Dcol], F32, tag="wtmp")
                nc.vector.tensor_tensor(out=tmp[:], in0=Wt[:], in1=ab[:],
                                        op=mybir.AluOpType.mult)
                red32 = wpool.tile([Fin, hh], F32, tag="wred")
                nc.vector.tensor_reduce(
                    out=red32[:],
                    in_=tmp[:].rearrange("p (h ch) -> p h ch", h=hh),
                    axis=mybir.AxisListType.X, op=mybir.AluOpType.add)
                nc.vector.tensor_copy(
                    out=aug[:, Dcol + j * hh:Dcol + (j + 1) * hh], in_=red32[:])
            if l >= 2:
                ps = ppC.tile([Fin, Dcol + 2 * hh], F32, space="PSUM", tag="misc")
                nc.tensor.matmul(ps[:], lhsT=PsigT[:], rhs=aug[:], start=True, stop=True)
                aug2 = cpool.tile([Fin, Dcol + 2 * hh], F16, tag=f"waugR{l}")
                nc.vector.tensor_copy(out=aug2[:], in_=ps[:])
                return aug2
            aug2 = cpool.tile([Fin, Dcol + 2 * hh], F16, tag=f"waugR{l}")
            nc.vector.tensor_copy(out=aug2[:], in_=aug[:])
            return aug2

        WA = {1: build_waug(1, W1, c.F, c.D, c.H),
              2: build_waug(2, W2, c.D, c.D, c.H),
              3: build_waug(3, W3, c.D, c.HID, 1)}

        # bias broadcast tiles (R-permuted for l<3), only if nonzero
        bias_t = {}
        for l, bp, wid in ((1, b1, c.D), (2, b2, c.D), (3, b3, c.HID)):
            if nonzero_bias[l - 1]:
                bt = cpool.tile([P, wid], F32, tag=f"bias{l}")
                if l < 3:
                    nc.sync.dma_start(
                        out=bt[:].rearrange("p (ch h) -> p ch h", ch=c.HID),
                        in_=bp[0:1, :].rearrange("o (h ch) -> o h ch", h=c.H)
                            .transpose([0, 2, 1]).to_broadcast([P, c.HID, c.H]))
                else:
                    nc.sync.dma_start(out=bt[:], in_=bp[0:1, :].to_broadcast([P, wid]))
                bias_t[l] = bt

        # ---------------- fused phase-1 (projection for own nodes) ----------
        def phase1_block(l, h_f16, b):
            """h_f16: [128, Fin] fp16 tile of this core's nodes (R layout for
            l-1>=1). Computes xl_aug for layer l; writes agin rows (DRAM) and
            the a_d table column (SBUF)."""
            n0 = b * P
            nb = min(P, c.NSH - n0)
            hh = c.H if l < 3 else 1
            wid = (c.D if l < 3 else c.HID) + 2 * hh
            hT = opool.tile([P, P], F16, tag="p1hT")
            nc.sync.dma_start_transpose(out=hT[:], in_=h_f16)
            xp = ppB.tile([P, wid], F32, space="PSUM", tag="p1xl")
            nc.tensor.matmul(xp[:], lhsT=hT[:], rhs=WA[l][:], start=True, stop=True)
            nf = wid - hh          # feats + a_s
            xf = opool.tile([P, nf], F16, tag="p1xf")
            nc.vector.tensor_copy(out=xf[:], in_=xp[:, 0:nf])
            nc.vector.tensor_copy(out=adS[l][:, b * 8:b * 8 + hh], in_=xp[:, nf:wid])
            nc.sync.dma_start(out=agin[l][n0:n0 + nb, 0:nf], in_=xf[0:nb, :])

        # block index after whose phase-1 the first table half is complete;
        # the AG itself is emitted two chunks later so its engine-block
        # overlaps queued gather generation
        AG_SPLIT_BLK = (HSH - 1) // P
        AG_EMIT_CHUNK = AG_SPLIT_BLK // cfg.CB + 3

        # ---------------- per-layer edge phase ----------------
        def allgather(l, h):
            r0 = h * HSH
            nc.gpsimd.collective_compute(
                "AllGather", mybir.AluOpType.bypass, replica_groups=rg,
                ins=[agin[l][r0:r0 + HSH, :].opt()],
                outs=[xltab[(l, h)].ap().opt()])

        gq = [0]  # round-robin queue counter for gathers

        def edge_layer(l):
            """Edge phase for layer l: queue-parallel feature gathers, on-chip
            one-hot build, a_d broadcast via S0^T matmuls, segment-softmax,
            per-block one-hot scatter matmuls accumulated in PSUM; fuses layer
            l+1's projection (l<3) or the pooling accumulation (l==3). The
            next layer's first-half AllGather fires as soon as its table rows
            are written (mid-layer), overlapping the collective with the
            remaining chunks."""
            hh = c.H if l < 3 else 1
            nfeat = c.D if l < 3 else c.HID
            rowlen = ROW12 if l < 3 else ROW3
            ascol = nfeat
            rhsw = nfeat + hh          # weighted feats + p (normalizer col(s))
            pooled = None
            if l == 3:
                pooled = ppC.tile([c.G, c.HID + 1], F32, space="PSUM", tag="misc")

            def gather_a(ch, gt):
                Tlo = ch['Tlo']
                if not Tlo:
                    return
                il = ipool.tile([P, 8 * Tlo], I16, tag="ilo")
                nc.sync.dma_start(out=il[:], in_=idx_lo[:, ch['col_lo']:ch['col_lo'] + 8 * Tlo])
                nc.gpsimd.dma_gather(
                    out_ap=gt[:, 0:Tlo, :], in_ap=xltab[(l, 0)][0:HN, :],
                    idxs_ap=il[:],
                    num_idxs=Tlo * P, num_idxs_reg=Tlo * P, elem_size=rowlen,
                    single_packet=False, queue_num=1 + gq[0] % NQ)
                gq[0] += 1

            # prefetch the first-half gathers of the leading chunks; their
            # descriptor generation overlaps the second-half AllGather's
            # engine-block below
            pre = {}
            for ki in range(min(4, len(ch_list))):
                ch = ch_list[ki]
                gt = gpool.tile([P, ch['T'], rowlen], F16, tag="gath")
                gather_a(ch, gt)
                pre[ki] = gt
            allgather(l, 1)

            def front(ki):
                """Gathers + one-hot loads + a_d matmuls for chunk ki —
                emitted one chunk ahead of back() so the PE/DVE never stall
                behind the gather-data dependency of the previous chunk."""
                ch = ch_list[ki]
                T, Tlo, Thi = ch['T'], ch['Tlo'], ch['Thi']
                c0 = ch['col_dst']
                if ki in pre:
                    gt = pre.pop(ki)
                else:
                    gt = gpool.tile([P, T, rowlen], F16, tag="gath")
                    gather_a(ch, gt)
                if l < 3 and ki == AG_EMIT_CHUNK:
                    allgather(l + 1, 0)
                if Thi:
                    ih = ipool.tile([P, 8 * Thi], I16, tag="ihi")
                    nc.sync.dma_start(out=ih[:], in_=idx_hi[:, ch['col_hi']:ch['col_hi'] + 8 * Thi])
                    nc.gpsimd.dma_gather(
                        out_ap=gt[:, Tlo:T, :], in_ap=xltab[(l, 1)][0:HN, :],
                        idxs_ap=ih[:],
                        num_idxs=Thi * P, num_idxs_reg=Thi * P, elem_size=rowlen,
                        single_packet=False, queue_num=1 + gq[0] % NQ)
                    gq[0] += 1

                s0tc = spool.tile([P, T * P], F16, tag="s0t")
                nc.sync.dma_start(out=s0tc[:], in_=s0t_d[:, P * c0:P * (c0 + T)])
                s0c = spool.tile([P, T, P], F16, tag="s0")
                nc.sync.dma_start(out=s0c[:], in_=s0_d[:, P * c0:P * (c0 + T)])

                # a_d broadcast to edge slots: ps_ad[p, t*hh+j] via S0^T matmul
                ps_ad = ppAd.tile([P, T * hh], F32, space="PSUM", tag="ad")
                for bi in ch['blocks']:
                    b = bi['b']
                    for t0, tn in (bi['lo'], bi['hi']):
                        for t in range(t0, t0 + tn):
                            nc.tensor.matmul(
                                ps_ad[:, t * hh:(t + 1) * hh],
                                lhsT=s0tc[:, P * t:P * (t + 1)],
                                rhs=adS[l][:, b * 8:b * 8 + hh],
                                start=True, stop=True)
                return ch, gt, s0c, ps_ad

            def back(st):
                ch, gt, s0c, ps_ad = st
                T, Tlo, Thi = ch['T'], ch['Tlo'], ch['Thi']

                # softmax weights: p = exp(lrelu(a_s[src] + a_d[dst]))
                eb = epool.tile([P, T, hh], F16, tag="ebuf")
                nc.vector.tensor_tensor(
                    out=eb[:], in0=gt[:, :, ascol:ascol + hh],
                    in1=ps_ad[:].rearrange("p (t j) -> p t j", t=T),
                    op=mybir.AluOpType.add)
                e2 = epool.tile([P, T, hh], F16, tag="ebuf2")
                nc.vector.tensor_scalar_mul(e2[:], eb[:], NEG_SLOPE)
                nc.vector.tensor_tensor(out=eb[:], in0=eb[:], in1=e2[:],
                                        op=mybir.AluOpType.max)
                # chunk-wide rhs: [p * feats | p]; exp writes its p values
                # directly into the rhs normalizer columns
                rhsc = rpool.tile([P, T, rhsw], F16, tag="rhs")
                nc.scalar.activation(rhsc[:, :, nfeat:rhsw], eb[:],
                                     mybir.ActivationFunctionType.Exp)
                pb = rhsc[:, :, nfeat:rhsw]
                if l < 3:
                    nc.vector.tensor_tensor(
                        out=rhsc[:, :, 0:nfeat].rearrange("p t (ch h) -> p t ch h", h=c.H),
                        in0=gt[:, :, 0:nfeat].rearrange("p t (ch h) -> p t ch h", h=c.H),
                        in1=pb.unsqueeze(2).to_broadcast([P, T, c.HID, c.H]),
                        op=mybir.AluOpType.mult)
                else:
                    nc.vector.tensor_tensor(
                        out=rhsc[:, :, 0:nfeat], in0=gt[:, :, 0:nfeat],
                        in1=pb[:, :, 0:1].to_broadcast([P, T, nfeat]),
                        op=mybir.AluOpType.mult)

                # per-block one-hot matmul accumulation
                nblk = len(ch['blocks'])
                accs = []
                for bi in ch['blocks']:
                    acc = ppA.tile([P, rhsw], F32, space="PSUM", tag="acc")
                    accs.append(acc)
                    tiles = (list(range(bi['lo'][0], bi['lo'][0] + bi['lo'][1]))
                             + list(range(bi['hi'][0], bi['hi'][0] + bi['hi'][1])))
                    for it, t in enumerate(tiles):
                        nc.tensor.matmul(acc[:], lhsT=s0c[:, t, :],
                                         rhs=rhsc[:, t, :],
                                         start=(it == 0), stop=(it == len(tiles) - 1))

                # batched post-processing for the chunk's blocks
                hcs = opool.tile([P, nblk, rhsw], F32, tag="hc")
                for j, acc in enumerate(accs):
                    nc.vector.tensor_copy(out=hcs[:, j, :], in_=acc[:])
                sr = opool.tile([P, nblk, hh], F32, tag="srecip")
                nc.vector.tensor_scalar_add(sr[:], hcs[:, :, nfeat:rhsw], EPS)
                nc.vector.reciprocal(sr[:], sr[:])
                if l < 3:
                    hn = opool.tile([P, nblk, nfeat], F32, tag="hnorm")
                    nc.vector.tensor_tensor(
                        out=hn[:].rearrange("p b (ch h) -> p b ch h", h=c.H),
                        in0=hcs[:, :, 0:nfeat].rearrange("p b (ch h) -> p b ch h", h=c.H),
                        in1=sr[:].unsqueeze(2).to_broadcast([P, nblk, c.HID, c.H]),
                        op=mybir.AluOpType.mult)
                    if l in bias_t:
                        nc.vector.tensor_tensor(
                            out=hn[:], in0=hn[:],
                            in1=bias_t[l][:].unsqueeze(1).to_broadcast([P, nblk, nfeat]),
                            op=mybir.AluOpType.add)
                    # elu(x) = (exp(min(x,0)) - 1) + max(x, 0)
                    mn = opool.tile([P, nblk, nfeat], F32, tag="emin")
                    nc.vector.tensor_scalar_min(mn[:], hn[:], 0.0)
                    em = opool.tile([P, nblk, nfeat], F32, tag="eexp")
                    nc.scalar.activation(em[:], mn[:], mybir.ActivationFunctionType.Exp)
                    rl = opool.tile([P, nblk, nfeat], F32, tag="erelu")
                    nc.vector.tensor_scalar_max(rl[:], hn[:], 0.0)
                    nc.vector.tensor_tensor(out=em[:], in0=em[:], in1=rl[:],
                                            op=mybir.AluOpType.add)
                    for j, bi in enumerate(ch['blocks']):
                        hf = opool.tile([P, nfeat], F16, tag="hfin")
                        nc.vector.tensor_scalar_add(hf[:], em[:, j, :], -1.0)
                        phase1_block(l + 1, hf[:], bi['b'])
                else:
                    h3 = opool.tile([P, nblk, c.HID + 1], F16, tag="h3r")
                    hn3 = opool.tile([P, nblk, c.HID], F32, tag="h3n")
                    nc.vector.tensor_tensor(
                        out=hn3[:], in0=hcs[:, :, 0:nfeat],
                        in1=sr[:, :, 0:1].to_broadcast([P, nblk, nfeat]),
                        op=mybir.AluOpType.mult)
                    if l in bias_t:
                        nc.vector.tensor_tensor(
                            out=hn3[:], in0=hn3[:],
                            in1=bias_t[3][:].unsqueeze(1).to_broadcast([P, nblk, c.HID]),
                            op=mybir.AluOpType.add)
                    nc.vector.tensor_scalar_max(h3[:, :, 0:c.HID], hn3[:], 0.0)
                    nc.vector.tensor_copy(
                        out=h3[:, :, c.HID:c.HID + 1],
                        in_=ones16[:].unsqueeze(1).to_broadcast([P, nblk, 1]))
                    for j, bi in enumerate(ch['blocks']):
                        b = bi['b']
                        nc.tensor.matmul(pooled[:],
                                         lhsT=gsel_t[:, b * c.G:(b + 1) * c.G],
                                         rhs=h3[:, j, :],
                                         start=(b == 0), stop=(b == c.NBLK - 1))

            nch = len(ch_list)
            sts = [front(ki) for ki in range(min(2, nch))]
            for ki in range(nch):
                if ki + 2 < nch:
                    sts.append(front(ki + 2))
                back(sts.pop(0))
            return pooled

        gsel_t = cpool.tile([P, c.NBLK * c.G], F16)
        nc.sync.dma_start(out=gsel_t[:], in_=gsel_d[:, :])

        # agin pad columns (136:256 / 17:128) are gathered but never read by
        # any compute op, so they are left unwritten.

        # ---------------- layer-1 phase-1 from x ----------------
        for b0 in range(0, c.NBLK, 4):
            bn = min(4, c.NBLK - b0)
            n0 = b0 * P
            nb = min(4 * P, c.NSH - n0)
            xt = opool.tile([P, 4, c.F], F16, tag="p1x")
            if nb < 4 * P:
                nc.vector.memset(xt[:], 0.0)
                full = nb // P
                if full:
                    nc.gpsimd.dma_start(
                        out=xt[:, 0:full, :],
                        in_=x_shard[n0:n0 + full * P, :].rearrange(
                            "(j p) f -> p j f", p=P))
                rem = nb - full * P
                if rem:
                    nc.gpsimd.dma_start(out=xt[0:rem, full, :],
                                        in_=x_shard[n0 + full * P:n0 + nb, :])
            else:
                nc.gpsimd.dma_start(
                    out=xt[:],
                    in_=x_shard[n0:n0 + 4 * P, :].rearrange(
                        "(j p) f -> p j f", p=P))
            for j in range(bn):
                phase1_block(1, xt[:, j, :], b0 + j)
                if b0 + j == AG_SPLIT_BLK:
                    allgather(1, 0)
        edge_layer(1)
        edge_layer(2)
        pooled = edge_layer(3)

        # ---------------- finale: AllReduce + linear head ----------------
        pl = opool.tile([c.G, c.HID + 1], F32, tag="plsb")
        nc.vector.tensor_copy(out=pl[:], in_=pooled[:])
        nc.sync.dma_start(out=arin[:, :], in_=pl[:])
        nc.gpsimd.collective_compute(
            "AllReduce", mybir.AluOpType.add, replica_groups=rg,
            ins=[arin.ap().opt()], outs=[arout.ap().opt()])
        pa = opool.tile([c.G, c.HID + 1], F32, tag="plall")
        nc.sync.dma_start(out=pa[:], in_=arout[:, :])
        cr = opool.tile([c.G, 1], F32, tag="cnt")
        nc.vector.tensor_scalar_max(cr[:], pa[:, c.HID:c.HID + 1], 1.0)
        nc.vector.reciprocal(cr[:], cr[:])
        pm = opool.tile([c.G, c.HID], F32, tag="pmean")
        nc.vector.tensor_scalar(out=pm[:], in0=pa[:, 0:c.HID], scalar1=cr[:, 0:1],
                                scalar2=None, op0=mybir.AluOpType.mult)
        tps = ppC.tile([c.HID, c.G], F32, space="PSUM", tag="misc")
        nc.tensor.transpose(out=tps[:], in_=pm[:], identity=ident32[:])
        lhs = opool.tile([c.HID + 1, c.G], F32, tag="flhs")
        nc.gpsimd.memset(lhs[:], 1.0)
        nc.vector.tensor_copy(out=lhs[0:c.HID, :], in_=tps[:])
        wl = cpool.tile([c.HID + 1, c.OUT], F32, tag="wlin")
        nc.sync.dma_start(out=wl[0:c.HID, :], in_=W_lin[:, :])
        nc.sync.dma_start(out=wl[c.HID:c.HID + 1, :], in_=b_lin[:, :])
        ops_ = ppC.tile([c.G, c.OUT], F32, space="PSUM", tag="misc")
        nc.tensor.matmul(ops_[:], lhsT=lhs[:], rhs=wl[:], start=True, stop=True)
        ot = opool.tile([c.G, c.OUT], F32, tag="osb")
        nc.vector.tensor_copy(out=ot[:], in_=ops_[:])
        nc.sync.dma_start(out=out[:, :], in_=ot[:])
    return nc


def make_in_maps(inputs, cfg, meta, arrays):
    c = cfg
    base = {
        'W1': np.asarray(inputs['W1'], np.float32),
        'W2': np.asarray(inputs['W2'], np.float32),
        'W3': np.asarray(inputs['W3'], np.float32),
        'b1': np.asarray(inputs['b1'], np.float32).reshape(1, -1),
        'b2': np.asarray(inputs['b2'], np.float32).reshape(1, -1),
        'b3': np.asarray(inputs['b3'], np.float32).reshape(1, -1),
        'W_lin': np.asarray(inputs['W_lin'], np.float32),
        'b_lin': np.asarray(inputs['b_lin'], np.float32).reshape(1, -1),
        'iota128': np.arange(P, dtype=np.float16).reshape(1, P),
        'iota64': np.arange(c.G, dtype=np.float16).reshape(1, c.G),
        'sigma': _sigma_perm(c).astype(np.float32).reshape(P, 1),
    }
    for l in (1, 2, 3):
        base[f'att_src{l}'] = np.asarray(inputs[f'att_src{l}'], np.float32)
        base[f'att_dst{l}'] = np.asarray(inputs[f'att_dst{l}'], np.float32)
    x = np.asarray(inputs['x'], np.float32)
    maps = []
    for ci in range(c.CORES):
        m = dict(base)
        m['x_shard'] = x[ci * c.NSH:(ci + 1) * c.NSH]
        m.update(arrays[ci])
        maps.append(m)
    return maps


_CACHE = {}


def run(inputs, trace=False):
    cfg = Cfg()
    key = ('v2',)
    if key not in _CACHE:
        meta, arrays = preprocess(np.asarray(inputs['edge_index']),
                                  np.asarray(inputs['batch']), cfg)
        nonzero_bias = [bool(np.any(np.asarray(inputs[k]))) for k in ('b1', 'b2', 'b3')]
        nc = bacc.Bacc(num_swdge_queues=4)
        build_program(nc, cfg, meta, nonzero_bias)
        nc.compile()
        _CACHE[key] = (nc, meta, arrays)
    nc, meta, arrays = _CACHE[key]
    maps = make_in_maps(inputs, cfg, meta, arrays)
    from concourse.bass_utils import run_bass_kernel_spmd
    res = run_bass_kernel_spmd(nc, maps, core_ids=list(range(cfg.CORES)),
                               trace=trace)
    return res.results[0]['out'].astype(np.float32), res.exec_time_ns


def kernel(**inputs):
    return run(inputs)[0]
